# revision 34
# baseline (speedup 1.0000x reference)
"""Trainium2 Bass kernel for masked candidate-span attention (ragged_sequence).

Math (per char n):
  s_v = x_n . M_v  for all v in [0,96)   with M = pos_embed @ W  [96, 512]
  masked softmax over the 9 candidates collapses to v-space with
  multiplicities: w_v = cnt_v * exp(s_v) = exp(s_v + ln cnt_v),
  Z = sum_v w_v, ctx = (w @ pos_embed) / Z.

Work split:
  Host (numpy, cheap): per-core slicing; x -> xT f16 [512, 8192];
  cnt histogram of masked candidate indices -> lncnt = ln(cnt) f16
  (0 -> -60000 sentinel, exact zero after exp; no-candidate rows get
  lncnt = 0 and are zeroed on the host afterwards together with the
  l >= seq_len padding); MT = (pos_embed @ W)^T f16; pos_ext bf16
  [96, 129] with a ones column so the ctx matmul also yields Z.

  Device, per 512-char supertile (16 per core), all v-major [96v, n]:
    PE:   s = MT^T @ xT (4 f16 matmuls, K=128 chunks) accumulated with
          lncnt injected via an identity matmul -> psum [96, 512]
    Act:  w = exp(s + lncnt) -> bf16 (single activation, no separate
          cnt multiply)
    PE:   ctx|Z psum [128n, (j, 129)] = w_j^T @ pos_ext (4 bf16 matmuls)
    DVE:  rz = 1/Z from psum; normalize ctx * rz -> f16 out (Act helps
          with one slice)
    DMA:  xs + lncnt tiles in on the SP queue, f16 out on the GpSimd
          queue; output upcast to f32 on the host.
Sharding: pure data parallel over batch (2 batches per core x 8 cores).
"""
import os
import sys

import numpy as np

sys.path.insert(0, "/opt/trn_rl_repo")
_HERE = os.path.dirname(os.path.abspath(__file__))
sys.path.insert(0, _HERE)

from contextlib import ExitStack

import concourse.bass as bass  # noqa: E402
import concourse.mybir as mybir  # noqa: E402
from concourse.tile import TileContext  # noqa: E402

# --- walrus workaround: cap sync waits per instruction ---------------------
import concourse.tile as _tile_mod  # noqa: E402
import bass_rust as _br  # noqa: E402
from concourse.vector_clock import ScopedClock  # noqa: E402


def _patched_drain_and_barrier(self, tick_clock, wait_clock):
    nc = self.nc
    probe = mybir.InstNoOp(name=nc.get_next_instruction_name(), ins=[], outs=[])
    probe.engine = mybir.EngineType.SP
    wait_clock.add_sem_waits(probe, ScopedClock({None: tick_clock.global_clock}))
    waits = list(probe.sync_info.on_wait)
    assert self.sems is not None
    by_num = {h.num: h for h in self.sems.allocated().values()}
    for w in waits:
        nc.sync.wait_ge(by_num[w.id], w.wait_value)
    nc.sync.drain()
    nc.all_engine_barrier()
    popped = nc._tile_sem_poison_stack.pop()
    assert popped is self._sem_poison
    nc.clear_and_free_semaphores(list(self.sems.allocated().values()))
    nc.all_engine_barrier()


_tile_mod.TileContext._drain_and_barrier = _patched_drain_and_barrier


def split_excess_waits(nc):
    for f in nc.m.functions:
        for bb in f.blocks:
            out = []
            changed = False
            for inst in bb.instructions:
                si = inst.sync_info
                waits = list(si.on_wait) if si is not None else []
                cap = 2 if isinstance(inst, _br.InstEventSemaphore) else 1
                if len(waits) > cap:
                    excess, keep = waits[:-cap], waits[-cap:]
                    for k in range(0, len(excess), 2):
                        ev = _br.InstEventSemaphore(
                            name=f"{inst.name}-wsplit{k}", ins=[], outs=[])
                        ev.engine = inst.engine
                        ev.sync_info = _br.SyncInfo(on_wait=excess[k:k + 2],
                                                    on_update=[])
                        out.append(ev)
                    inst.sync_info = _br.SyncInfo(on_wait=keep,
                                                  on_update=list(si.on_update))
                    changed = True
                out.append(inst)
            if changed:
                bb.instructions = out


# --- problem constants -----------------------------------------------------
B, L, C = 16, 4096, 9
DI, DO, V = 512, 128, 96
NCORES = 8
BLOC = B // NCORES          # batches per core
NLOC = BLOC * L             # chars per core (8192)
NSUP = NLOC // 512          # 16 supertiles (512 chars each)
VSPLIT = 48                 # DVE handles v < VSPLIT, Pool v >= VSPLIT
ZEPS = 1e-33

f32 = mybir.dt.float32
f16 = mybir.dt.float16
bf16 = mybir.dt.bfloat16
i32 = mybir.dt.int32
i16 = mybir.dt.int16
Alu = mybir.AluOpType
Act = mybir.ActivationFunctionType
Ax = mybir.AxisListType


def build_kernel():
    nc = bass.Bass()
    xt_d = nc.declare_dram_parameter("xt", [DI, NLOC], f16, isOutput=False)
    cnt_d = nc.declare_dram_parameter("lncnt", [V, NLOC], f16,
                                      isOutput=False)
    id96_d = nc.declare_dram_parameter("ident96", [V, V], f16,
                                       isOutput=False)
    mt_d = nc.declare_dram_parameter("mt", [DI, V], f16, isOutput=False)
    pos_d = nc.declare_dram_parameter("pos_ext", [V, DO + 1], bf16,
                                      isOutput=False)
    out_d = nc.declare_dram_parameter("out", [NSUP * 128, 4 * DO], f16,
                                      isOutput=True)

    with TileContext(nc) as tc, ExitStack() as es:
        cpool = es.enter_context(tc.tile_pool(name="consts", bufs=1))
        # ---- constants ----
        # ---- weights ----
        mt_sb = cpool.tile([128, 4 * V], f16)      # [128d, (k, 96v)]
        nc.sync.dma_start(
            out=mt_sb[:].rearrange("p (k v) -> p k v", k=4),
            in_=mt_d[:].rearrange("(k p) v -> p k v", p=128))
        pos_sb = cpool.tile([V, DO + 1], bf16)
        nc.sync.dma_start(out=pos_sb[:], in_=pos_d[:])
        id96_sb = cpool.tile([V, V], f16)
        nc.sync.dma_start(out=id96_sb[:], in_=id96_d[:])
        # preload the Exp activation table while preamble DMAs run
        warm = cpool.tile([128, 1], f32)
        nc.scalar.activation(out=warm[:], in_=mt_sb[:, 0:1], func=Act.Exp,
                             bias=0.0, scale=1.0)

        # ---- pools ----
        xpool = es.enter_context(tc.tile_pool(name="x", bufs=2))
        ipool = es.enter_context(tc.tile_pool(name="ix", bufs=3))
        epool = es.enter_context(tc.tile_pool(name="soft", bufs=8))
        qpool = es.enter_context(tc.tile_pool(name="cntv", bufs=2))
        wpool = es.enter_context(tc.tile_pool(name="wv", bufs=5))
        opool = es.enter_context(tc.tile_pool(name="outp", bufs=3))
        ps_s = es.enter_context(tc.tile_pool(name="ps_s", bufs=2, space="PSUM"))
        ps_cx = es.enter_context(tc.tile_pool(name="ps_cx", bufs=6, space="PSUM"))

        for pr in range(NSUP // 4):
            n0 = pr * 2048
            xs = xpool.tile([128, 4 * 2048], f16, tag="xs")
            nc.sync.dma_start(
                out=xs[:].rearrange("p (k n) -> p k n", k=4),
                in_=xt_d[:, n0:n0 + 2048].rearrange("(k p) n -> p k n",
                                                    p=128))
            ct = ipool.tile([V, 2048], f16, tag="ct")
            nc.gpsimd.dma_start(out=ct[:], in_=cnt_d[:, n0:n0 + 2048])
            outsb = opool.tile([128, 4 * 512], f16, tag="outsb")

            # phase A: score matmuls for all halves (keeps PE dense)
            psts = []
            for h2 in range(4):
                x0 = h2 * 512
                pst = ps_s.tile([V, 512], f32, tag="pst")
                for k in range(4):
                    nc.tensor.matmul(
                        pst[:], mt_sb[:, k * V:(k + 1) * V],
                        xs[:, k * 2048 + x0:k * 2048 + x0 + 512],
                        start=(k == 0), stop=False)
                nc.tensor.matmul(pst[:], id96_sb[:], ct[:, x0:x0 + 512],
                                 start=False, stop=True)
                psts.append(pst)
            # phase B: w = exp(s + ln cnt) for both halves
            ws = []
            for h2 in range(4):
                w = wpool.tile([V, 512], bf16, tag="w")
                with nc.allow_low_precision("w bf16 ok (normalized later)"):
                    nc.scalar.activation(out=w[:], in_=psts[h2][:],
                                         func=Act.Exp, bias=0.0, scale=1.0)
                ws.append(w)
            # phase C: ctx | Z matmuls for both halves
            pcss = []
            for h2 in range(4):
                pcs = []
                for h in range(2):
                    pc = ps_cx.tile([128, 2 * (DO + 1)], f32, tag="pc")
                    for jj in range(2):
                        j = 2 * h + jj
                        nc.tensor.matmul(
                            pc[:, jj * (DO + 1):(jj + 1) * (DO + 1)],
                            ws[h2][:, j * 128:(j + 1) * 128],
                            pos_sb[:], start=True, stop=True)
                    pcs.append(pc)
                pcss.append(pcs)
            # phase D: reciprocal + normalize for both halves
            for h2 in range(4):
                x0 = h2 * 512
                pcs = pcss[h2]
                rz = epool.tile([128, 4], f32, tag="rz")
                for h in range(2):
                    pa = pcs[h][:]
                    zin = bass.AP(pa.tensor, pa.offset + DO,
                                  [pa.ap[0], [DO + 1, 2]])
                    nc.vector.reciprocal(rz[:, 2 * h:2 * h + 2], zin)
                pa = pcs[0][:]
                cin = bass.AP(pa.tensor, pa.offset,
                              [pa.ap[0], [DO + 1, 2], [1, DO]])
                ra = rz[:, 0:1]
                rin = bass.AP(ra.tensor, ra.offset,
                              [ra.ap[0], [1, 2], [0, DO]])
                oa = outsb[:, x0:x0 + 2 * DO]
                oout = bass.AP(oa.tensor, oa.offset,
                               [oa.ap[0], [DO, 2], [1, DO]])
                nc.vector.tensor_tensor(out=oout, in0=cin, in1=rin,
                                        op=Alu.mult)
                nc.vector.tensor_scalar(
                    out=outsb[:, x0 + 2 * DO:x0 + 3 * DO],
                    in0=pcs[1][:, 0:DO],
                    scalar1=rz[:, 2:3], scalar2=None, op0=Alu.mult)
                nc.scalar.activation(
                    out=outsb[:, x0 + 3 * DO:x0 + 4 * DO],
                    in_=pcs[1][:, (DO + 1):(DO + 1) + DO],
                    func=Act.Copy, bias=0.0, scale=rz[:, 3:4])

            nc.gpsimd.dma_start(
                out=out_d[pr * 512:(pr + 1) * 512, :].rearrange(
                    "(s p) o -> p s o", p=128),
                in_=outsb[:].rearrange("p (s o) -> p s o", s=4))

    split_excess_waits(nc)
    return nc


_NC_CACHE = None


def make_in_map(inputs, b0):
    import ml_dtypes
    x = np.asarray(inputs["input_context"][b0:b0 + BLOC],
                   np.float32).reshape(NLOC, DI)
    idx = np.asarray(inputs["cand_idx"][b0:b0 + BLOC],
                     np.int32).reshape(NLOC, C)
    msk = np.asarray(inputs["cand_mask"][b0:b0 + BLOC]).reshape(NLOC, C)
    W = np.asarray(inputs["W"], np.float32)
    pos = np.asarray(inputs["pos_embed"], np.float32)

    xt = np.ascontiguousarray(x.T).astype(np.float16)
    lin = (np.arange(NLOC, dtype=np.int64)[:, None] * V + idx).ravel()
    lin = lin[msk.ravel().astype(bool)]
    cnt = np.bincount(lin, minlength=NLOC * V).reshape(NLOC, V)
    has_cand = cnt.any(axis=1)
    with np.errstate(divide="ignore"):
        lncnt = np.where(cnt > 0, np.log(np.maximum(cnt, 1)), -60000.0)
    lncnt[~has_cand] = 0.0
    lncnt = np.ascontiguousarray(lncnt.T).astype(np.float16)
    mt = np.ascontiguousarray((pos @ W).T).astype(np.float16)
    ident96 = np.eye(V, dtype=np.float16)
    pos_ext = np.concatenate(
        [pos, np.ones((V, 1), np.float32)], axis=1).astype(ml_dtypes.bfloat16)
    return {"xt": xt, "lncnt": lncnt, "mt": mt, "pos_ext": pos_ext,
            "ident96": ident96}, has_cand


def kernel(**inputs):
    global _NC_CACHE
    from concourse.bass_utils import run_bass_kernel_spmd

    if _NC_CACHE is None:
        _NC_CACHE = build_kernel()
    nc = _NC_CACHE

    packed = [make_in_map(inputs, c * BLOC) for c in range(NCORES)]
    in_maps = [p[0] for p in packed]
    has_cand = np.stack([p[1] for p in packed]).reshape(B, L)
    res = run_bass_kernel_spmd(nc, in_maps, core_ids=list(range(NCORES)))
    slen = np.asarray(inputs["word_seq_len"], np.int32)
    out = np.empty((B, L, DO), np.float32)
    for c in range(NCORES):
        o = res.results[c]["out"].astype(np.float32).reshape(NSUP, 128, 4, DO)
        o = o.transpose(0, 2, 1, 3).reshape(BLOC, L, DO)
        out[c * BLOC:(c + 1) * BLOC] = o
    inlen = np.arange(L, dtype=np.int32)[None, :] < slen[:, None]
    out *= (inlen & has_cand)[:, :, None]
    return out


# revision 35
# speedup vs baseline: 1.0843x; 1.0843x over previous
"""Trainium2 Bass kernel for masked candidate-span attention (ragged_sequence).

Math (per char n):
  s_v = x_n . M_v  for all v in [0,96)   with M = pos_embed @ W  [96, 512]
  masked softmax over the 9 candidates collapses to v-space with
  multiplicities: w_v = cnt_v * exp(s_v) = exp(s_v + ln cnt_v),
  Z = sum_v w_v, ctx = (w @ pos_embed) / Z.

Work split:
  Host (numpy, cheap): per-core slicing; x -> xT f16 [512, 8192];
  cnt histogram of masked candidate indices -> lncnt = ln(cnt) f16
  (0 -> -60000 sentinel, exact zero after exp; no-candidate rows get
  lncnt = 0 and are zeroed on the host afterwards together with the
  l >= seq_len padding); MT = (pos_embed @ W)^T f16; pos_ext bf16
  [96, 129] with a ones column so the ctx matmul also yields Z.

  Device, per 512-char supertile (16 per core), all v-major [96v, n]:
    PE:   s = MT^T @ xT (4 f16 matmuls, K=128 chunks) accumulated with
          lncnt injected via an identity matmul -> psum [96, 512]
    Act:  w = exp(s + lncnt) -> bf16 (single activation, no separate
          cnt multiply)
    PE:   ctx|Z psum [128n, (j, 129)] = w_j^T @ pos_ext (4 bf16 matmuls)
    DVE:  rz = 1/Z from psum; normalize ctx * rz -> f16 out (Act helps
          with one slice)
    DMA:  xs + lncnt tiles in on the SP queue, f16 out on the GpSimd
          queue; output upcast to f32 on the host.
Sharding: pure data parallel over batch (2 batches per core x 8 cores).
"""
import os
import sys

import numpy as np

sys.path.insert(0, "/opt/trn_rl_repo")
_HERE = os.path.dirname(os.path.abspath(__file__))
sys.path.insert(0, _HERE)

from contextlib import ExitStack

import concourse.bass as bass  # noqa: E402
import concourse.mybir as mybir  # noqa: E402
from concourse.tile import TileContext  # noqa: E402

# --- walrus workaround: cap sync waits per instruction ---------------------
import concourse.tile as _tile_mod  # noqa: E402
import bass_rust as _br  # noqa: E402
from concourse.vector_clock import ScopedClock  # noqa: E402


def _patched_drain_and_barrier(self, tick_clock, wait_clock):
    nc = self.nc
    probe = mybir.InstNoOp(name=nc.get_next_instruction_name(), ins=[], outs=[])
    probe.engine = mybir.EngineType.SP
    wait_clock.add_sem_waits(probe, ScopedClock({None: tick_clock.global_clock}))
    waits = list(probe.sync_info.on_wait)
    assert self.sems is not None
    by_num = {h.num: h for h in self.sems.allocated().values()}
    for w in waits:
        nc.sync.wait_ge(by_num[w.id], w.wait_value)
    nc.sync.drain()
    nc.all_engine_barrier()
    popped = nc._tile_sem_poison_stack.pop()
    assert popped is self._sem_poison
    nc.clear_and_free_semaphores(list(self.sems.allocated().values()))
    nc.all_engine_barrier()


_tile_mod.TileContext._drain_and_barrier = _patched_drain_and_barrier


def split_excess_waits(nc):
    for f in nc.m.functions:
        for bb in f.blocks:
            out = []
            changed = False
            for inst in bb.instructions:
                si = inst.sync_info
                waits = list(si.on_wait) if si is not None else []
                cap = 2 if isinstance(inst, _br.InstEventSemaphore) else 1
                if len(waits) > cap:
                    excess, keep = waits[:-cap], waits[-cap:]
                    for k in range(0, len(excess), 2):
                        ev = _br.InstEventSemaphore(
                            name=f"{inst.name}-wsplit{k}", ins=[], outs=[])
                        ev.engine = inst.engine
                        ev.sync_info = _br.SyncInfo(on_wait=excess[k:k + 2],
                                                    on_update=[])
                        out.append(ev)
                    inst.sync_info = _br.SyncInfo(on_wait=keep,
                                                  on_update=list(si.on_update))
                    changed = True
                out.append(inst)
            if changed:
                bb.instructions = out


# --- problem constants -----------------------------------------------------
B, L, C = 16, 4096, 9
DI, DO, V = 512, 128, 96
NCORES = 8
BLOC = B // NCORES          # batches per core
NLOC = BLOC * L             # chars per core (8192)
NSUP = NLOC // 512          # 16 supertiles (512 chars each)
VSPLIT = 48                 # DVE handles v < VSPLIT, Pool v >= VSPLIT
ZEPS = 1e-33

f32 = mybir.dt.float32
f16 = mybir.dt.float16
bf16 = mybir.dt.bfloat16
i32 = mybir.dt.int32
i16 = mybir.dt.int16
Alu = mybir.AluOpType
Act = mybir.ActivationFunctionType
Ax = mybir.AxisListType


def build_kernel():
    nc = bass.Bass()
    xt_d = nc.declare_dram_parameter("xt", [DI, NLOC], f16, isOutput=False)
    cnt_d = nc.declare_dram_parameter("lncnt", [V, NLOC], f16,
                                      isOutput=False)
    id96_d = nc.declare_dram_parameter("ident96", [V, V], f16,
                                       isOutput=False)
    mt_d = nc.declare_dram_parameter("mt", [DI, V], f16, isOutput=False)
    pos_d = nc.declare_dram_parameter("pos_ext", [V, DO + 1], bf16,
                                      isOutput=False)
    out_d = nc.declare_dram_parameter("out", [NSUP * 128, 4 * DO], f16,
                                      isOutput=True)

    with TileContext(nc) as tc, ExitStack() as es:
        cpool = es.enter_context(tc.tile_pool(name="consts", bufs=1))
        # ---- constants ----
        # ---- weights ----
        mt_sb = cpool.tile([128, 4 * V], f16)      # [128d, (k, 96v)]
        nc.sync.dma_start(
            out=mt_sb[:].rearrange("p (k v) -> p k v", k=4),
            in_=mt_d[:].rearrange("(k p) v -> p k v", p=128))
        pos_sb = cpool.tile([V, DO + 1], bf16)
        nc.sync.dma_start(out=pos_sb[:], in_=pos_d[:])
        id96_sb = cpool.tile([V, V], f16)
        nc.sync.dma_start(out=id96_sb[:], in_=id96_d[:])
        # preload the Exp activation table while preamble DMAs run
        warm = cpool.tile([128, 1], f32)
        nc.scalar.activation(out=warm[:], in_=mt_sb[:, 0:1], func=Act.Exp,
                             bias=0.0, scale=1.0)

        # ---- pools ----
        xpool = es.enter_context(tc.tile_pool(name="x", bufs=6))
        ipool = es.enter_context(tc.tile_pool(name="ix", bufs=6))
        epool = es.enter_context(tc.tile_pool(name="soft", bufs=8))
        qpool = es.enter_context(tc.tile_pool(name="cntv", bufs=2))
        wpool = es.enter_context(tc.tile_pool(name="wv", bufs=5))
        opool = es.enter_context(tc.tile_pool(name="outp", bufs=6))
        ps_s = es.enter_context(tc.tile_pool(name="ps_s", bufs=2, space="PSUM"))
        ps_cx = es.enter_context(tc.tile_pool(name="ps_cx", bufs=6, space="PSUM"))

        for pr in range(NSUP // 2):
            n0 = pr * 1024
            xs = xpool.tile([128, 4 * 1024], f16, tag="xs")
            nc.sync.dma_start(
                out=xs[:].rearrange("p (k n) -> p k n", k=4),
                in_=xt_d[:, n0:n0 + 1024].rearrange("(k p) n -> p k n",
                                                    p=128))
            ct = ipool.tile([V, 1024], f16, tag="ct")
            nc.gpsimd.dma_start(out=ct[:], in_=cnt_d[:, n0:n0 + 1024])
            outsb = opool.tile([128, 2 * 512], f16, tag="outsb")

            # phase A: score matmuls for both halves (keeps PE dense)
            psts = []
            for h2 in range(2):
                x0 = h2 * 512
                pst = ps_s.tile([V, 512], f32, tag="pst")
                for k in range(4):
                    nc.tensor.matmul(
                        pst[:], mt_sb[:, k * V:(k + 1) * V],
                        xs[:, k * 1024 + x0:k * 1024 + x0 + 512],
                        start=(k == 0), stop=False)
                nc.tensor.matmul(pst[:], id96_sb[:], ct[:, x0:x0 + 512],
                                 start=False, stop=True)
                psts.append(pst)
            # phase B: w = exp(s + ln cnt) for both halves
            ws = []
            for h2 in range(2):
                w = wpool.tile([V, 512], bf16, tag="w")
                with nc.allow_low_precision("w bf16 ok (normalized later)"):
                    nc.scalar.activation(out=w[:], in_=psts[h2][:],
                                         func=Act.Exp, bias=0.0, scale=1.0)
                ws.append(w)
            # phase C: ctx | Z matmuls for both halves
            pcss = []
            for h2 in range(2):
                pcs = []
                for h in range(2):
                    pc = ps_cx.tile([128, 2 * (DO + 1)], f32, tag="pc")
                    for jj in range(2):
                        j = 2 * h + jj
                        nc.tensor.matmul(
                            pc[:, jj * (DO + 1):(jj + 1) * (DO + 1)],
                            ws[h2][:, j * 128:(j + 1) * 128],
                            pos_sb[:], start=True, stop=True)
                    pcs.append(pc)
                pcss.append(pcs)
            # phase D: reciprocal + normalize for both halves
            for h2 in range(2):
                x0 = h2 * 512
                pcs = pcss[h2]
                rz = epool.tile([128, 4], f32, tag="rz")
                for h in range(2):
                    pa = pcs[h][:]
                    zin = bass.AP(pa.tensor, pa.offset + DO,
                                  [pa.ap[0], [DO + 1, 2]])
                    nc.vector.reciprocal(rz[:, 2 * h:2 * h + 2], zin)
                pa = pcs[0][:]
                cin = bass.AP(pa.tensor, pa.offset,
                              [pa.ap[0], [DO + 1, 2], [1, DO]])
                ra = rz[:, 0:1]
                rin = bass.AP(ra.tensor, ra.offset,
                              [ra.ap[0], [1, 2], [0, DO]])
                oa = outsb[:, x0:x0 + 2 * DO]
                oout = bass.AP(oa.tensor, oa.offset,
                               [oa.ap[0], [DO, 2], [1, DO]])
                nc.vector.tensor_tensor(out=oout, in0=cin, in1=rin,
                                        op=Alu.mult)
                nc.vector.tensor_scalar(
                    out=outsb[:, x0 + 2 * DO:x0 + 3 * DO],
                    in0=pcs[1][:, 0:DO],
                    scalar1=rz[:, 2:3], scalar2=None, op0=Alu.mult)
                nc.scalar.activation(
                    out=outsb[:, x0 + 3 * DO:x0 + 4 * DO],
                    in_=pcs[1][:, (DO + 1):(DO + 1) + DO],
                    func=Act.Copy, bias=0.0, scale=rz[:, 3:4])

            nc.gpsimd.dma_start(
                out=out_d[pr * 256:(pr + 1) * 256, :].rearrange(
                    "(s p) o -> p s o", p=128),
                in_=outsb[:].rearrange("p (s o) -> p s o", s=2))

    split_excess_waits(nc)
    return nc


_NC_CACHE = None


def make_in_map(inputs, b0):
    import ml_dtypes
    x = np.asarray(inputs["input_context"][b0:b0 + BLOC],
                   np.float32).reshape(NLOC, DI)
    idx = np.asarray(inputs["cand_idx"][b0:b0 + BLOC],
                     np.int32).reshape(NLOC, C)
    msk = np.asarray(inputs["cand_mask"][b0:b0 + BLOC]).reshape(NLOC, C)
    W = np.asarray(inputs["W"], np.float32)
    pos = np.asarray(inputs["pos_embed"], np.float32)

    xt = np.ascontiguousarray(x.T).astype(np.float16)
    lin = (np.arange(NLOC, dtype=np.int64)[:, None] * V + idx).ravel()
    lin = lin[msk.ravel().astype(bool)]
    cnt = np.bincount(lin, minlength=NLOC * V).reshape(NLOC, V)
    has_cand = cnt.any(axis=1)
    with np.errstate(divide="ignore"):
        lncnt = np.where(cnt > 0, np.log(np.maximum(cnt, 1)), -60000.0)
    lncnt[~has_cand] = 0.0
    lncnt = np.ascontiguousarray(lncnt.T).astype(np.float16)
    mt = np.ascontiguousarray((pos @ W).T).astype(np.float16)
    ident96 = np.eye(V, dtype=np.float16)
    pos_ext = np.concatenate(
        [pos, np.ones((V, 1), np.float32)], axis=1).astype(ml_dtypes.bfloat16)
    return {"xt": xt, "lncnt": lncnt, "mt": mt, "pos_ext": pos_ext,
            "ident96": ident96}, has_cand


def kernel(**inputs):
    global _NC_CACHE
    from concourse.bass_utils import run_bass_kernel_spmd

    if _NC_CACHE is None:
        _NC_CACHE = build_kernel()
    nc = _NC_CACHE

    packed = [make_in_map(inputs, c * BLOC) for c in range(NCORES)]
    in_maps = [p[0] for p in packed]
    has_cand = np.stack([p[1] for p in packed]).reshape(B, L)
    res = run_bass_kernel_spmd(nc, in_maps, core_ids=list(range(NCORES)))
    slen = np.asarray(inputs["word_seq_len"], np.int32)
    out = np.empty((B, L, DO), np.float32)
    for c in range(NCORES):
        o = res.results[c]["out"].astype(np.float32).reshape(NSUP, 128, 4, DO)
        o = o.transpose(0, 2, 1, 3).reshape(BLOC, L, DO)
        out[c * BLOC:(c + 1) * BLOC] = o
    inlen = np.arange(L, dtype=np.int32)[None, :] < slen[:, None]
    out *= (inlen & has_cand)[:, :, None]
    return out


# revision 36
# speedup vs baseline: 1.2345x; 1.1385x over previous
"""Trainium2 Bass kernel for masked candidate-span attention (ragged_sequence).

Math (per char n):
  s_v = x_n . M_v  for all v in [0,96)   with M = pos_embed @ W  [96, 512]
  masked softmax over the 9 candidates collapses to v-space with
  multiplicities: w_v = cnt_v * exp(s_v) = exp(s_v + ln cnt_v),
  Z = sum_v w_v, ctx = (w @ pos_embed) / Z.

Work split:
  Host (numpy, cheap): per-core slicing; x -> xT f16 [512, 8192];
  cnt histogram of masked candidate indices -> lncnt = ln(cnt) f16
  (0 -> -60000 sentinel, exact zero after exp; no-candidate rows get
  lncnt = 0 and are zeroed on the host afterwards together with the
  l >= seq_len padding); MT = (pos_embed @ W)^T f16; pos_ext bf16
  [96, 129] with a ones column so the ctx matmul also yields Z.

  Device, per 512-char supertile (16 per core), all v-major [96v, n]:
    PE:   s = MT^T @ xT (4 f16 matmuls, K=128 chunks) accumulated with
          lncnt injected via an identity matmul -> psum [96, 512]
    Act:  w = exp(s + lncnt) -> bf16 (single activation, no separate
          cnt multiply)
    PE:   ctx|Z psum [128n, (j, 129)] = w_j^T @ pos_ext (4 bf16 matmuls)
    DVE:  rz = 1/Z from psum; normalize ctx * rz -> f16 out (Act helps
          with one slice)
    DMA:  xs + lncnt tiles in on the SP queue, f16 out on the GpSimd
          queue; output upcast to f32 on the host.
Sharding: pure data parallel over batch (2 batches per core x 8 cores).
"""
import os
import sys

import numpy as np

sys.path.insert(0, "/opt/trn_rl_repo")
_HERE = os.path.dirname(os.path.abspath(__file__))
sys.path.insert(0, _HERE)

from contextlib import ExitStack

import concourse.bass as bass  # noqa: E402
import concourse.mybir as mybir  # noqa: E402
from concourse.tile import TileContext  # noqa: E402

# --- walrus workaround: cap sync waits per instruction ---------------------
import concourse.tile as _tile_mod  # noqa: E402
import bass_rust as _br  # noqa: E402
from concourse.vector_clock import ScopedClock  # noqa: E402


def _patched_drain_and_barrier(self, tick_clock, wait_clock):
    nc = self.nc
    probe = mybir.InstNoOp(name=nc.get_next_instruction_name(), ins=[], outs=[])
    probe.engine = mybir.EngineType.SP
    wait_clock.add_sem_waits(probe, ScopedClock({None: tick_clock.global_clock}))
    waits = list(probe.sync_info.on_wait)
    assert self.sems is not None
    by_num = {h.num: h for h in self.sems.allocated().values()}
    for w in waits:
        nc.sync.wait_ge(by_num[w.id], w.wait_value)
    nc.sync.drain()
    nc.all_engine_barrier()
    popped = nc._tile_sem_poison_stack.pop()
    assert popped is self._sem_poison
    nc.clear_and_free_semaphores(list(self.sems.allocated().values()))
    nc.all_engine_barrier()


_tile_mod.TileContext._drain_and_barrier = _patched_drain_and_barrier


def split_excess_waits(nc):
    for f in nc.m.functions:
        for bb in f.blocks:
            out = []
            changed = False
            for inst in bb.instructions:
                si = inst.sync_info
                waits = list(si.on_wait) if si is not None else []
                cap = 2 if isinstance(inst, _br.InstEventSemaphore) else 1
                if len(waits) > cap:
                    excess, keep = waits[:-cap], waits[-cap:]
                    for k in range(0, len(excess), 2):
                        ev = _br.InstEventSemaphore(
                            name=f"{inst.name}-wsplit{k}", ins=[], outs=[])
                        ev.engine = inst.engine
                        ev.sync_info = _br.SyncInfo(on_wait=excess[k:k + 2],
                                                    on_update=[])
                        out.append(ev)
                    inst.sync_info = _br.SyncInfo(on_wait=keep,
                                                  on_update=list(si.on_update))
                    changed = True
                out.append(inst)
            if changed:
                bb.instructions = out


# --- problem constants -----------------------------------------------------
B, L, C = 16, 4096, 9
DI, DO, V = 512, 128, 96
NCORES = 8
BLOC = B // NCORES          # batches per core
NLOC = BLOC * L             # chars per core (8192)
NSUP = NLOC // 512          # 16 supertiles (512 chars each)
VSPLIT = 48                 # DVE handles v < VSPLIT, Pool v >= VSPLIT
ZEPS = 1e-33

f32 = mybir.dt.float32
f16 = mybir.dt.float16
bf16 = mybir.dt.bfloat16
i32 = mybir.dt.int32
i16 = mybir.dt.int16
Alu = mybir.AluOpType
Act = mybir.ActivationFunctionType
Ax = mybir.AxisListType


def build_kernel():
    nc = bass.Bass()
    xt_d = nc.declare_dram_parameter("xt", [DI, NLOC], f16, isOutput=False)
    cnt_d = nc.declare_dram_parameter("lncnt", [V, NLOC], f16,
                                      isOutput=False)
    id96_d = nc.declare_dram_parameter("ident96", [V, V], f16,
                                       isOutput=False)
    mt_d = nc.declare_dram_parameter("mt", [DI, V], f16, isOutput=False)
    pos_d = nc.declare_dram_parameter("pos_ext", [V, DO + 1], bf16,
                                      isOutput=False)
    out_d = nc.declare_dram_parameter("out", [NSUP * 128, 4 * DO], f16,
                                      isOutput=True)

    with TileContext(nc) as tc, ExitStack() as es:
        cpool = es.enter_context(tc.tile_pool(name="consts", bufs=1))
        # ---- constants ----
        # ---- weights ----
        mt_sb = cpool.tile([128, 4 * V], f16)      # [128d, (k, 96v)]
        nc.sync.dma_start(
            out=mt_sb[:].rearrange("p (k v) -> p k v", k=4),
            in_=mt_d[:].rearrange("(k p) v -> p k v", p=128))
        pos_sb = cpool.tile([V, DO + 1], bf16)
        nc.sync.dma_start(out=pos_sb[:], in_=pos_d[:])
        id96_sb = cpool.tile([V, V], f16)
        nc.sync.dma_start(out=id96_sb[:], in_=id96_d[:])
        # preload the Exp activation table while preamble DMAs run
        warm = cpool.tile([128, 1], f32)
        nc.scalar.activation(out=warm[:], in_=mt_sb[:, 0:1], func=Act.Exp,
                             bias=0.0, scale=1.0)

        # ---- pools ----
        xpool = es.enter_context(tc.tile_pool(name="x", bufs=6))
        ipool = es.enter_context(tc.tile_pool(name="ix", bufs=6))
        epool = es.enter_context(tc.tile_pool(name="soft", bufs=8))
        qpool = es.enter_context(tc.tile_pool(name="cntv", bufs=2))
        wpool = es.enter_context(tc.tile_pool(name="wv", bufs=5))
        opool = es.enter_context(tc.tile_pool(name="outp", bufs=6))
        ps_s = es.enter_context(tc.tile_pool(name="ps_s", bufs=2, space="PSUM"))
        ps_cx = es.enter_context(tc.tile_pool(name="ps_cx", bufs=6, space="PSUM"))

        for pr in range(NSUP // 2):
            n0 = pr * 1024
            xs = xpool.tile([128, 4 * 1024], f16, tag="xs")
            xsv = xs[:].rearrange("p (k n) -> p k n", k=4)
            src = xt_d[:, n0:n0 + 1024]
            nc.sync.dma_start(
                out=xsv[:, 0:2, :],
                in_=src[0:256, :].rearrange("(k p) n -> p k n", p=128))
            nc.scalar.dma_start(
                out=xsv[:, 2:4, :],
                in_=src[256:512, :].rearrange("(k p) n -> p k n", p=128))
            ct = ipool.tile([V, 1024], f16, tag="ct")
            nc.gpsimd.dma_start(out=ct[:], in_=cnt_d[:, n0:n0 + 1024])
            outsb = opool.tile([128, 2 * 512], f16, tag="outsb")

            # phase A: score matmuls for both halves (keeps PE dense)
            psts = []
            for h2 in range(2):
                x0 = h2 * 512
                pst = ps_s.tile([V, 512], f32, tag="pst")
                for k in range(4):
                    nc.tensor.matmul(
                        pst[:], mt_sb[:, k * V:(k + 1) * V],
                        xs[:, k * 1024 + x0:k * 1024 + x0 + 512],
                        start=(k == 0), stop=False)
                nc.tensor.matmul(pst[:], id96_sb[:], ct[:, x0:x0 + 512],
                                 start=False, stop=True)
                psts.append(pst)
            # phase B: w = exp(s + ln cnt) for both halves
            ws = []
            for h2 in range(2):
                w = wpool.tile([V, 512], bf16, tag="w")
                with nc.allow_low_precision("w bf16 ok (normalized later)"):
                    nc.scalar.activation(out=w[:], in_=psts[h2][:],
                                         func=Act.Exp, bias=0.0, scale=1.0)
                ws.append(w)
            # phase C: ctx | Z matmuls for both halves
            pcss = []
            for h2 in range(2):
                pcs = []
                for h in range(2):
                    pc = ps_cx.tile([128, 2 * (DO + 1)], f32, tag="pc")
                    for jj in range(2):
                        j = 2 * h + jj
                        nc.tensor.matmul(
                            pc[:, jj * (DO + 1):(jj + 1) * (DO + 1)],
                            ws[h2][:, j * 128:(j + 1) * 128],
                            pos_sb[:], start=True, stop=True)
                    pcs.append(pc)
                pcss.append(pcs)
            # phase D: reciprocal + normalize for both halves
            for h2 in range(2):
                x0 = h2 * 512
                pcs = pcss[h2]
                rz = epool.tile([128, 4], f32, tag="rz")
                for h in range(2):
                    pa = pcs[h][:]
                    zin = bass.AP(pa.tensor, pa.offset + DO,
                                  [pa.ap[0], [DO + 1, 2]])
                    nc.vector.reciprocal(rz[:, 2 * h:2 * h + 2], zin)
                pa = pcs[0][:]
                cin = bass.AP(pa.tensor, pa.offset,
                              [pa.ap[0], [DO + 1, 2], [1, DO]])
                ra = rz[:, 0:1]
                rin = bass.AP(ra.tensor, ra.offset,
                              [ra.ap[0], [1, 2], [0, DO]])
                oa = outsb[:, x0:x0 + 2 * DO]
                oout = bass.AP(oa.tensor, oa.offset,
                               [oa.ap[0], [DO, 2], [1, DO]])
                nc.vector.tensor_tensor(out=oout, in0=cin, in1=rin,
                                        op=Alu.mult)
                nc.vector.tensor_scalar(
                    out=outsb[:, x0 + 2 * DO:x0 + 3 * DO],
                    in0=pcs[1][:, 0:DO],
                    scalar1=rz[:, 2:3], scalar2=None, op0=Alu.mult)
                nc.scalar.activation(
                    out=outsb[:, x0 + 3 * DO:x0 + 4 * DO],
                    in_=pcs[1][:, (DO + 1):(DO + 1) + DO],
                    func=Act.Copy, bias=0.0, scale=rz[:, 3:4])

            nc.gpsimd.dma_start(
                out=out_d[pr * 256:(pr + 1) * 256, :].rearrange(
                    "(s p) o -> p s o", p=128),
                in_=outsb[:].rearrange("p (s o) -> p s o", s=2))

    split_excess_waits(nc)
    return nc


_NC_CACHE = None


def make_in_map(inputs, b0):
    import ml_dtypes
    x = np.asarray(inputs["input_context"][b0:b0 + BLOC],
                   np.float32).reshape(NLOC, DI)
    idx = np.asarray(inputs["cand_idx"][b0:b0 + BLOC],
                     np.int32).reshape(NLOC, C)
    msk = np.asarray(inputs["cand_mask"][b0:b0 + BLOC]).reshape(NLOC, C)
    W = np.asarray(inputs["W"], np.float32)
    pos = np.asarray(inputs["pos_embed"], np.float32)

    xt = np.ascontiguousarray(x.T).astype(np.float16)
    lin = (np.arange(NLOC, dtype=np.int64)[:, None] * V + idx).ravel()
    lin = lin[msk.ravel().astype(bool)]
    cnt = np.bincount(lin, minlength=NLOC * V).reshape(NLOC, V)
    has_cand = cnt.any(axis=1)
    with np.errstate(divide="ignore"):
        lncnt = np.where(cnt > 0, np.log(np.maximum(cnt, 1)), -60000.0)
    lncnt[~has_cand] = 0.0
    lncnt = np.ascontiguousarray(lncnt.T).astype(np.float16)
    mt = np.ascontiguousarray((pos @ W).T).astype(np.float16)
    ident96 = np.eye(V, dtype=np.float16)
    pos_ext = np.concatenate(
        [pos, np.ones((V, 1), np.float32)], axis=1).astype(ml_dtypes.bfloat16)
    return {"xt": xt, "lncnt": lncnt, "mt": mt, "pos_ext": pos_ext,
            "ident96": ident96}, has_cand


def kernel(**inputs):
    global _NC_CACHE
    from concourse.bass_utils import run_bass_kernel_spmd

    if _NC_CACHE is None:
        _NC_CACHE = build_kernel()
    nc = _NC_CACHE

    packed = [make_in_map(inputs, c * BLOC) for c in range(NCORES)]
    in_maps = [p[0] for p in packed]
    has_cand = np.stack([p[1] for p in packed]).reshape(B, L)
    res = run_bass_kernel_spmd(nc, in_maps, core_ids=list(range(NCORES)))
    slen = np.asarray(inputs["word_seq_len"], np.int32)
    out = np.empty((B, L, DO), np.float32)
    for c in range(NCORES):
        o = res.results[c]["out"].astype(np.float32).reshape(NSUP, 128, 4, DO)
        o = o.transpose(0, 2, 1, 3).reshape(BLOC, L, DO)
        out[c * BLOC:(c + 1) * BLOC] = o
    inlen = np.arange(L, dtype=np.int32)[None, :] < slen[:, None]
    out *= (inlen & has_cand)[:, :, None]
    return out
